# revision 22
# baseline (speedup 1.0000x reference)
"""Conv3d(16->64, k=3, VALID) + sigmoid(tanh(conv*scale)*bias), B=8 sharded
over 8 NeuronCores (one batch element per core).

v4 scheme (per core): bf16 4-pass matmuls (fp8 DoubleRow measured 2x
slower per element on this toolchain), single-tanh epilogue, packed
bf16 stores.

  - x pre-cast to bf16 on host, shift-replicated: xa rows = 8 (kd,kw)
    combos x 16 ci (all but (2,2)), kh as free-dim offsets -> 3 passes
    of K=128; x48 rows = (kh,ci) -> one K=48 pass covers tap (2,*,2).
  - psum [128 = (half, co), 32 h, 64 w] per plane, pool bufs=2; kh-outer
    across PLANE PAIRS so consecutive matmuls share lhsT -> 2 weight
    loads per plane instead of 4.
  - Epilogue: sigmoid(b*tanh(a)) ~= 0.5 + beta*tanh(gamma*a), beta =
    sigmoid(b)-1/2, gamma = b/(4 beta): one ACT Tanh pass over the
    valid [128, 32, 62] region (scale=gamma per partition), one DVE
    tensor_scalar affine (bf16 4x mode), two packed bf16 stores
    (partitions 0:64 h<32, 64:128 h 32..61) -> out [64, 62^3] with NO
    host-side crop; host upcasts bf16 -> fp32.
"""

import sys

sys.path.insert(0, "/opt/trn_rl_repo")

import numpy as np
import ml_dtypes

import concourse.bass as bass
import concourse.mybir as mybir
from concourse import tile
from concourse.bass_utils import run_bass_kernel_spmd

# ---- problem constants (hardcoded per spec) ----
B = 8
CIN = 16
COUT = 64
K = 3
S = 64  # input spatial
SO = S - K + 1  # 62 output spatial
PLANE = S * S  # 4096
HALF = PLANE // 2  # 2048
NCORES = 8
OPLANE = SO * SO  # 3844 packed output plane
OHALF = 32 * SO  # 1984 packed elements from partitions 0:64 (h 0..31)

CHUNK = 4  # output planes per window load

PAD_PLANES = 7
XPLANES = S + PAD_PLANES
XSTRIDE = XPLANES * PLANE  # per-channel row stride in xr

WINA = CHUNK * PLANE + 160  # xa window elements per partition
WIN48 = CHUNK * PLANE + 64  # x48 window elements per partition


def split_multiwaits(nc):
    """walrus in this toolchain rejects instructions carrying more than one
    sync-wait. Rewrite every multi-wait instruction into (n-1) single-wait
    nops on the same engine queue followed by the instruction with the last
    wait — identical semantics since each engine queue executes serially."""
    for func in nc.m.functions:
        for block in func.blocks:
            insts = block.instructions
            if not any(
                i.sync_info is not None and len(i.sync_info.on_wait or ()) > 1
                for i in insts
            ):
                continue
            newlist = []
            for inst in insts:
                si = inst.sync_info
                if si is not None and si.on_wait and len(si.on_wait) > 1:
                    waits = list(si.on_wait)
                    for w in waits[:-1]:
                        nop = mybir.InstNoOp(
                            name=nc.get_next_instruction_name(),
                            sync_info=mybir.SyncInfo(on_wait=[w], on_update=[]),
                            bass_nofuse=True,
                            engine=inst.engine,
                        )
                        newlist.append(nop)
                    si.on_wait = waits[-1:]
                newlist.append(inst)
            insts[:] = newlist


class PatchedTileContext(tile.TileContext):
    def __exit__(self, exc_type, exc_value, traceback):
        ret = super().__exit__(exc_type, exc_value, traceback)
        if exc_type is None:
            split_multiwaits(self.nc)
        return ret


def build_nc(nplanes=SO, repeat=1, ablate=()):
    nc = bass.Bass(trn_type="TRN2")
    # host-replicated x: row (kd*48 + kw*16 + ci) = x[ci] shifted kd*PLANE+kw
    xr = nc.dram_tensor(
        "xr", [128, XSTRIDE], mybir.dt.bfloat16, kind="ExternalInput"
    )
    # host-replicated x: row (kh*16 + ci) = x[ci] shifted kh*S
    xr48 = nc.dram_tensor(
        "xr48", [K * CIN, XSTRIDE], mybir.dt.bfloat16, kind="ExternalInput"
    )
    wla = nc.dram_tensor("wla", [128, K * COUT], mybir.dt.bfloat16, kind="ExternalInput")
    wl48 = nc.dram_tensor("wl48", [K * CIN, COUT], mybir.dt.bfloat16, kind="ExternalInput")
    gvec = nc.dram_tensor("gvec", [2 * COUT, 1], mybir.dt.float32, kind="ExternalInput")
    bvec = nc.dram_tensor("bvec", [2 * COUT, 1], mybir.dt.float32, kind="ExternalInput")
    # packed valid output, bf16; host upcasts to fp32
    out = nc.dram_tensor(
        "out", [COUT, SO * OPLANE], mybir.dt.bfloat16, kind="ExternalOutput"
    )

    fp32 = mybir.dt.float32
    bf16 = mybir.dt.bfloat16
    AF = mybir.ActivationFunctionType

    with PatchedTileContext(nc) as tc:
        with (
            tc.tile_pool(name="const", bufs=1) as cpool,
            tc.tile_pool(name="xwina", bufs=2) as xapool,
            tc.tile_pool(name="xwin48", bufs=2) as x48pool,
            tc.tile_pool(name="eptmp", bufs=3) as epool,
            tc.tile_pool(name="outp", bufs=3) as opool,
            tc.tile_pool(name="psum", bufs=2, space="PSUM") as pspool,
        ):
            wla_sb = cpool.tile([128, K * COUT], bf16)
            wl48_sb = cpool.tile([K * CIN, COUT], bf16)
            gv_sb = cpool.tile([2 * COUT, 1], fp32)
            bv_sb = cpool.tile([2 * COUT, 1], fp32)
            nc.sync.dma_start(wla_sb[:], wla[:])
            nc.sync.dma_start(wl48_sb[:], wl48[:])
            nc.sync.dma_start(gv_sb[:], gvec[:])
            nc.sync.dma_start(bv_sb[:], bvec[:])

            def run_chunk(d0, ndp):
                xa = xapool.tile([128, WINA], bf16, tag="xa", name="xa")
                x48 = x48pool.tile([K * CIN, WIN48], bf16, tag="x48", name="x48")
                wl_ld = 64 if "loads" in ablate else WINA
                wl_ld48 = 64 if "loads" in ablate else WIN48
                nc.sync.dma_start(
                    xa[:, 0:wl_ld], xr[:, d0 * PLANE : d0 * PLANE + wl_ld]
                )
                nc.gpsimd.dma_start(
                    x48[:, 0:wl_ld48],
                    xr48[:, (d0 + 2) * PLANE : (d0 + 2) * PLANE + wl_ld48],
                )

                # plane pairs: kh-outer over the pair halves the lhsT loads
                for dl0 in range(0, ndp, 2):
                    dls = [dl for dl in (dl0, dl0 + 1) if dl < ndp]
                    pss = [
                        pspool.tile([2 * COUT, 32, S], fp32, tag="ps", name="ps")
                        for _ in dls
                    ]
                    if "mm" in ablate:
                        for ps in pss:
                            nc.tensor.matmul(
                                ps[0:COUT, 0:8, :],
                                wla_sb[:, 0:COUT],
                                xa[:, 0:512],
                                start=True,
                                stop=True,
                            )
                    else:
                        # psum partition p = half*64 + co; free = h_local*64+w
                        # (half innermost: consecutive matmuls alternate PE
                        # column groups)
                        # the last block of half 1 covers h 56..63; h 62,63
                        # are garbage, so stream only 6 of its 8 h-rows
                        def blk(half, b):
                            return 6 if (half == 1 and b == 3) else 8

                        for kh in range(K):
                            for ps, dl in zip(pss, dls):
                                for b in range(4):
                                    for half in range(2):
                                        po = half * COUT
                                        col = half * HALF + b * 512
                                        oa = dl * PLANE + kh * S + col
                                        nh = blk(half, b)
                                        nc.tensor.matmul(
                                            ps[po : po + COUT, b * 8 : b * 8 + nh, :],
                                            wla_sb[:, kh * COUT : (kh + 1) * COUT],
                                            xa[:, oa : oa + nh * S],
                                            start=(kh == 0),
                                            stop=False,
                                        )
                        for ps, dl in zip(pss, dls):
                            for b in range(4):
                                for half in range(2):
                                    po = half * COUT
                                    col = half * HALF + b * 512
                                    o48 = dl * PLANE + 2 + col
                                    nh = blk(half, b)
                                    nc.tensor.matmul(
                                        ps[po : po + COUT, b * 8 : b * 8 + nh, :],
                                        wl48_sb[:],
                                        x48[:, o48 : o48 + nh * S],
                                        start=False,
                                        stop=True,
                                    )
                    for ps, dl in zip(pss, dls):
                        d = d0 + dl
                        # epilogue reads the valid [128][32 h][62 w] strided
                        # region of psum and writes packed flat [128, 1984];
                        # partitions 64:128 cols 1860:1984 are garbage
                        # (h=62,63), skipped by the second store
                        t_sb = epool.tile([2 * COUT, OHALF], bf16, tag="t", name="t")
                        o_sb = opool.tile([2 * COUT, OHALF], bf16, tag="o", name="o")
                        nact = 512 if "act" in ablate else OHALF
                        act_in = (
                            ps[:, 0:8, 0:S] if "act" in ablate
                            else ps[:, :, 0:SO]
                        )
                        nc.scalar.activation(
                            t_sb[:, 0:nact], act_in, AF.Tanh, scale=gv_sb[:]
                        )
                        if "dve" in ablate:
                            o_sb = t_sb
                        else:
                            nc.vector.tensor_scalar(
                                o_sb[:, 0:nact],
                                t_sb[:, 0:nact],
                                bv_sb[:],
                                0.5,
                                mybir.AluOpType.mult,
                                mybir.AluOpType.add,
                            )
                        ws0 = 64 if "stores" in ablate else OHALF
                        ws1 = 64 if "stores" in ablate else OPLANE - OHALF
                        nc.scalar.dma_start(
                            out[:, d * OPLANE : d * OPLANE + ws0],
                            o_sb[0:COUT, 0:ws0],
                        )
                        nc.sync.dma_start(
                            out[:, d * OPLANE + OHALF : d * OPLANE + OHALF + ws1],
                            o_sb[COUT : 2 * COUT, 0:ws1],
                        )

            for _ in range(repeat):
                for d0 in range(0, nplanes, CHUNK):
                    run_chunk(d0, min(CHUNK, nplanes - d0))
    return nc


def _sigmoid(z):
    return 1.0 / (1.0 + np.exp(-z))


def prepare_in_maps(x, weight, scale, bias):
    x = np.asarray(x, dtype=np.float32)
    weight = np.asarray(weight, dtype=np.float32)
    scale = np.asarray(scale, dtype=np.float32)
    bias = np.asarray(bias, dtype=np.float32)

    # fold scale into weights
    w_eff = weight * scale.reshape(COUT, 1, 1, 1, 1)  # [co, ci, kd, kh, kw]

    # wla: [128 = (kd,kw,ci) packed, kh*64 + co]
    wla = np.zeros((128, K * COUT), dtype=np.float32)
    wt = w_eff.transpose(2, 4, 1, 3, 0)  # [kd, kw, ci, kh, co]
    wla[0:96] = wt[0:2].reshape(96, K * COUT)
    wla[96:128] = wt[2, 0:2].reshape(32, K * COUT)
    wla = wla.astype(ml_dtypes.bfloat16)

    # wl48: [48 = (kh,ci), co] for tap (kd=2, kw=2)
    wl48 = np.ascontiguousarray(
        w_eff[:, :, 2, :, 2].transpose(2, 1, 0).reshape(K * CIN, COUT)
    ).astype(ml_dtypes.bfloat16)

    # epilogue vectors: out ~= 0.5 + beta * tanh(gamma * a)
    b = bias.reshape(COUT).astype(np.float64)
    beta = _sigmoid(b) - 0.5
    gamma = np.where(np.abs(b) < 1e-3, 1.0 + b * b / 12.0, b / (4.0 * beta))
    gv = np.tile(gamma.astype(np.float32), 2).reshape(2 * COUT, 1)
    bv = np.tile(beta.astype(np.float32), 2).reshape(2 * COUT, 1)

    # host-side cast to bf16 + shift-replication into the matmul layouts
    xf = x.reshape(B, CIN, S * PLANE).astype(ml_dtypes.bfloat16)
    NTOT = S * PLANE
    xr = np.zeros((B, 128, XSTRIDE), dtype=ml_dtypes.bfloat16)
    for kd in range(K):
        for kw in range(K):
            if kd == 2 and kw == 2:
                continue
            p = kd * 48 + kw * 16 if kd < 2 else 96 + kw * 16
            sh = kd * PLANE + kw
            xr[:, p : p + CIN, : NTOT - sh] = xf[:, :, sh:]
    xr48 = np.zeros((B, K * CIN, XSTRIDE), dtype=ml_dtypes.bfloat16)
    for kh in range(K):
        sh = kh * S
        xr48[:, kh * CIN : (kh + 1) * CIN, : NTOT - sh] = xf[:, :, sh:]

    return [
        {
            "xr": xr[c],
            "xr48": xr48[c],
            "wla": wla,
            "wl48": wl48,
            "gvec": gv,
            "bvec": bv,
        }
        for c in range(NCORES)
    ]


_NC_CACHE = None
LAST_RESULT = None


def kernel(x, weight, scale, bias):
    global _NC_CACHE, LAST_RESULT
    in_maps = prepare_in_maps(x, weight, scale, bias)

    if _NC_CACHE is None:
        _NC_CACHE = build_nc()
    nc = _NC_CACHE

    res = run_bass_kernel_spmd(nc, in_maps, list(range(NCORES)))
    LAST_RESULT = res

    out = np.empty((B, COUT, SO, SO, SO), dtype=np.float32)
    for c in range(NCORES):
        out[c] = (
            res.results[c]["out"].astype(np.float32).reshape(COUT, SO, SO, SO)
        )
    return out
